# revision 10
# baseline (speedup 1.0000x reference)
"""CharLSTM (B=128, T=256, V=256, D=1024, L=4) on 8 trn2 NeuronCores.

Tensor-parallel over the 4*D gate dimension: core j owns, for each gate
m in {i,f,g,o}, columns [m*1024 + j*128 : m*1024 + (j+1)*128].  Hence
core j also owns h/c slice j*128:(j+1)*128 of the hidden dim.

Batch-major compute layout (activation-stationary matmuls): per layer
step, z[b, 512] = sum_k hT_chunk[k].T @ W_chunk[k] with N=512 moving
free dim, so the PE streams 512 cols per instruction (8 MMs per part
instead of 32 at N=128).  The LSTM cell runs elementwise on [b, gate]
tiles; the produced h slice [b,128] is transposed to [128 d, b] via the
DMA XBAR (off the PE), staged to DRAM, and AllGathered in layer pairs
(0,1) and (2,3) so every core has the full hT for the next step.

Layer l at time t runs at tick t + SKEW*l (wavefront), which gives the
x-part consumers >=1 tick of slack so AllGather latency hides under
matmuls of other layers.

Layer-0's x-part contracts a host-built one-hot over V=256 against
G0 = embed @ Wx[0] (computed on device), i.e. 2 matmuls instead of 8.

Output projection h3 @ Wout is computed redundantly on every core per
tick; the host reads core 0's copy.
"""

import numpy as np
import ml_dtypes

B, T, V, D, L = 128, 256, 256, 1024, 4
NCORES = 8
SKEW = 2
S = 4  # hbuf time slots
BF16 = ml_dtypes.bfloat16
TMODE = "xbar"  # "xbar" (DMA transpose) or "pe" (tensor-engine transpose)


def _host_prep(idx, embed, Wx, Wh, b, Wout, t_run):
    """Build per-core input maps (numpy)."""
    nw = (t_run + 3) // 4
    idx = np.asarray(idx)
    embed = np.asarray(embed, np.float32)
    Wx = np.asarray(Wx, np.float32)
    Wh = np.asarray(Wh, np.float32)
    b = np.asarray(b, np.float32)
    Wout = np.asarray(Wout, np.float32)
    assert not np.any(b), "nonzero bias not supported by this build"

    # embt[p, k, v] = embed[v, k*128+p]
    embt = np.ascontiguousarray(
        embed.T.reshape(8, 128, V).transpose(1, 0, 2)).astype(BF16)
    # wout[p, k, v] = Wout[k*128+p, v]
    wout = np.ascontiguousarray(
        Wout.reshape(8, 128, V).transpose(1, 0, 2)).astype(BF16)
    # one-hot: oh[w, p, c, kk, bb] = (idx[bb, 4w+kk] == c*128+p)
    ids = idx[:, :t_run]  # [B, t_run]
    onehot = (ids[None, :, :] == np.arange(V)[:, None, None])  # [V, B, t]
    oh_full = onehot.reshape(2, 128, B, nw, 4)  # [c, p, b, w, kk]
    oh = np.ascontiguousarray(
        oh_full.transpose(3, 1, 0, 4, 2)).astype(BF16)  # [w, p, c, kk, bb]

    # weight rhs layout: w[l, p, k, m*128+c] = W[l, k*128+p, m*1024+j*128+c]
    wx_full = Wx.reshape(L, 8, 128, 4, 8, 128)  # [l, k, p, m, j, c]
    wh_full = Wh.reshape(L, 8, 128, 4, 8, 128)

    in_maps = []
    for j in range(NCORES):
        wx_j = np.ascontiguousarray(
            wx_full[:, :, :, :, j, :].transpose(0, 2, 1, 3, 4)
        ).reshape(L, 128, 8, 512).astype(BF16)
        wh_j = np.ascontiguousarray(
            wh_full[:, :, :, :, j, :].transpose(0, 2, 1, 3, 4)
        ).reshape(L, 128, 8, 512).astype(BF16)
        in_map = {
            "wx": wx_j,
            "wh": wh_j,
            "embt": embt,
            "wout": wout,
            "oh": oh,
        }
        if TMODE == "pe":
            in_map["ident"] = np.eye(128, dtype=BF16)
        in_maps.append(in_map)
    return in_maps


def _build(nc, tile, mybir, t_run):
    """Emit the SPMD program for one core (identical on all cores)."""
    dt = mybir.dt
    nw = (t_run + 3) // 4

    wx_ext = nc.dram_tensor("wx", [L, 128, 8, 512], dt.bfloat16, kind="ExternalInput")
    wh_ext = nc.dram_tensor("wh", [L, 128, 8, 512], dt.bfloat16, kind="ExternalInput")
    embt_ext = nc.dram_tensor("embt", [128, 8, V], dt.bfloat16, kind="ExternalInput")
    wout_ext = nc.dram_tensor("wout", [128, 8, V], dt.bfloat16, kind="ExternalInput")
    oh_ext = nc.dram_tensor("oh", [nw, 128, 2, 4, 128], dt.bfloat16, kind="ExternalInput")
    if TMODE == "pe":
        ident_ext = nc.dram_tensor("ident", [128, 128], dt.bfloat16, kind="ExternalInput")
    out_ext = nc.dram_tensor("logits", [t_run, 128, V], dt.float32, kind="ExternalOutput")

    rg = [list(range(NCORES))]
    total_ticks = t_run + SKEW * (L - 1) + 1  # last tick flushes last proj

    with tile.TileContext(nc) as tc:
        with (
            tc.tile_pool(name="const", bufs=1) as cpool,
            tc.tile_pool(name="state", bufs=1) as spool,
            tc.tile_pool(name="work", bufs=6) as wpool,
            tc.tile_pool(name="ohp", bufs=2) as ohpool,
            tc.tile_pool(name="psum", bufs=6, space="PSUM") as psum,
            tc.tile_pool(name="ccin", bufs=3, space="DRAM") as ccin_pool,
            tc.tile_pool(name="ccout", bufs=3, space="DRAM") as ccout_pool,
        ):
            # ---- resident tiles ----
            wx_s = cpool.tile([128, L, 8, 512], dt.bfloat16)
            wh_s = cpool.tile([128, L, 8, 512], dt.bfloat16)
            embt_s = cpool.tile([128, 8, V], dt.bfloat16)
            wout_s = cpool.tile([128, 8, V], dt.bfloat16)
            g0_s = cpool.tile([128, 2, 512], dt.bfloat16)
            hbuf = spool.tile([128, L, 8, S, 128], dt.bfloat16)  # (d, l, k, slot, b)
            c_s = spool.tile([128, L, 128], dt.float32)

            for l in range(L):
                nc.sync.dma_start(wx_s[:, l], wx_ext[l])
                nc.sync.dma_start(wh_s[:, l], wh_ext[l])
            nc.sync.dma_start(embt_s[:], embt_ext[:])
            nc.sync.dma_start(wout_s[:], wout_ext[:])
            if TMODE == "pe":
                ident = cpool.tile([128, 128], dt.bfloat16)
                nc.sync.dma_start(ident[:], ident_ext[:])

            # ---- G0 = embed @ Wx[0] (slice), bf16, [v_chunk 128, 512] ----
            for c in range(2):
                pg = psum.tile([128, 512], dt.float32, tag="z", name="pg")
                for k in range(8):
                    nc.tensor.matmul(
                        pg[:], embt_s[:, k, c * 128:(c + 1) * 128], wx_s[:, 0, k],
                        start=(k == 0), stop=(k == 7),
                    )
                nc.vector.tensor_copy(g0_s[:, c], pg[:])

            # ---- main loop over ticks ----
            oh_tiles = {}
            for tau in range(total_ticks):
                active = [(l, tau - SKEW * l) for l in range(L)]
                active = [(l, t) for (l, t) in active if 0 <= t < t_run]
                act_map = dict(active)

                ccinA = ccin_pool.tile([2, 128, 128], dt.bfloat16, name="ccinA")
                ccinB = ccin_pool.tile([2, 128, 128], dt.bfloat16, name="ccinB")
                ccoutA = ccout_pool.tile([NCORES, 2, 128, 128], dt.bfloat16,
                                         addr_space="Shared", name="ccoutA")
                ccoutB = ccout_pool.tile([NCORES, 2, 128, 128], dt.bfloat16,
                                         addr_space="Shared", name="ccoutB")

                # one-hot window prefetch for layer 0 (2 ticks ahead of use)
                for w in range(nw):
                    if max(0, w * 4 - 2) == tau:
                        oht = ohpool.tile([128, 2, 4, 128], dt.bfloat16, name="oh_t")
                        nc.sync.dma_start(oht[:], oh_ext[w])
                        oh_tiles[w] = oht
                oh_t = oh_tiles.get(tau // 4)

                # ---- phase A: x-part matmuls of all active layers ----
                zps = {}
                for l, t in active:
                    zp = psum.tile([128, 512], dt.float32, name="zp", tag="z")
                    zps[l] = zp
                    if l == 0:
                        for c in range(2):
                            nc.tensor.matmul(
                                zp[:], oh_t[:, c, t % 4, :], g0_s[:, c],
                                start=(c == 0), stop=(t == 0 and c == 1),
                            )
                    else:
                        for k in range(8):
                            nc.tensor.matmul(
                                zp[:], hbuf[:, l - 1, k, t % S, :], wx_s[:, l, k],
                                start=(k == 0), stop=(t == 0 and k == 7),
                            )

                # ---- projection of layer-3 output (slot gathered >=1 tick ago) ----
                t3p = tau - SKEW * 3 - 1

                def emit_proj():
                    if not (0 <= t3p < t_run):
                        return
                    pp = psum.tile([128, V], dt.float32, name="pp", tag="pp", bufs=2)
                    for k in range(8):
                        nc.tensor.matmul(
                            pp[:], hbuf[:, 3, k, t3p % S, :], wout_s[:, k],
                            start=(k == 0), stop=(k == 7),
                        )
                    lg = wpool.tile([128, V], dt.float32, name="lg")
                    nc.vector.tensor_copy(lg[:], pp[:])
                    nc.scalar.dma_start(out_ext[t3p], lg[:])

                # ---- phase B: per layer h-part + cell + stage; post pair AGs ----
                lastA = max([l for l, _ in active if l < 2], default=None)
                lastB = max([l for l, _ in active if l >= 2], default=None)
                proj_done = False
                for l, t in active:
                    if l == 2:
                        emit_proj()
                        proj_done = True
                    zp = zps[l]
                    if t > 0:
                        for k in range(8):
                            nc.tensor.matmul(
                                zp[:], hbuf[:, l, k, (t - 1) % S, :], wh_s[:, l, k],
                                start=False, stop=(k == 7),
                            )

                    # ---- LSTM cell elementwise ([b, gate] layout) ----
                    # z slices: i=[0:128] f=[128:256] g=[256:384] o=[384:512]
                    sig = wpool.tile([128, 384], dt.float32, name="sig")
                    tg = wpool.tile([128, 128], dt.float32, name="tg")
                    nc.scalar.activation(
                        sig[:, 0:256], zp[:, 0:256],
                        mybir.ActivationFunctionType.Sigmoid)
                    nc.scalar.activation(
                        tg[:], zp[:, 256:384], mybir.ActivationFunctionType.Tanh)
                    nc.scalar.activation(
                        sig[:, 256:384], zp[:, 384:512],
                        mybir.ActivationFunctionType.Sigmoid)
                    ig = wpool.tile([128, 128], dt.float32, name="ig")
                    nc.vector.tensor_mul(ig[:], sig[:, 0:128], tg[:])
                    cv = c_s[:, l]
                    if t > 0:
                        nc.vector.tensor_mul(cv, cv, sig[:, 128:256])
                        nc.vector.tensor_add(cv, cv, ig[:])
                    else:
                        nc.vector.tensor_copy(cv, ig[:])
                    tch = wpool.tile([128, 128], dt.float32, name="tch")
                    nc.scalar.activation(tch[:], cv, mybir.ActivationFunctionType.Tanh)
                    h_sl = wpool.tile([128, 128], dt.bfloat16, name="h_sl")
                    nc.vector.tensor_mul(h_sl[:], sig[:, 256:384], tch[:])

                    # ---- transpose h slice to [d, b] and stage for gather ----
                    if TMODE == "xbar":
                        hT_sl = wpool.tile([128, 128], dt.bfloat16, name="hT_sl")
                        nc.sync.dma_start(hT_sl[:], h_sl[:], transpose=True)
                    else:
                        pt = psum.tile([128, 128], dt.bfloat16, name="pt", tag="pp",
                                       bufs=2)
                        nc.tensor.transpose(pt[:], h_sl[:], ident[:])
                        hT_sl = wpool.tile([128, 128], dt.bfloat16, name="hT_sl")
                        nc.vector.tensor_copy(hT_sl[:], pt[:])
                    nc.sync.dma_start((ccinA if l < 2 else ccinB)[l % 2], hT_sl[:])

                    # post the pair AllGather as soon as its layers are staged
                    if l == lastA or l == lastB:
                        ccin_p = ccinA if l == lastA else ccinB
                        ccout_p = ccoutA if l == lastA else ccoutB
                        nc.gpsimd.collective_compute(
                            "AllGather", mybir.AluOpType.bypass,
                            replica_groups=rg, ins=[ccin_p[:]], outs=[ccout_p[:]],
                        )
                        for lp in ((0, 1) if l == lastA else (2, 3)):
                            if lp not in act_map:
                                continue
                            tp = act_map[lp]
                            # ccout[r, lp%2, p, b] -> hbuf[p, lp, r, slot, b]
                            # (gpsimd SWDGE: keeps the AG-gated scatter off the
                            # in-order HWDGE queues so it can't block staging)
                            nc.gpsimd.dma_start(
                                hbuf[:, lp, :, tp % S, :],
                                ccout_p[:, lp % 2, :, :].transpose((1, 0, 2)),
                            )
                if not proj_done:
                    emit_proj()

    nc.compile()
    return nc


_CACHED = {}


def _get_nc(t_run):
    if t_run in _CACHED:
        return _CACHED[t_run]
    import concourse.bass as bass  # noqa: PLC0415
    import concourse.tile as tile  # noqa: PLC0415
    from concourse import bacc, mybir  # noqa: PLC0415

    nc = bacc.Bacc("TRN2", target_bir_lowering=False, debug=False,
                   num_devices=NCORES)
    _build(nc, tile, mybir, t_run)
    _CACHED[t_run] = nc
    return nc


def _postprocess(out, t_run):
    # out: [t, b, v] fp32 -> [B, t, V]
    return np.ascontiguousarray(
        np.asarray(out).transpose(1, 0, 2)).astype(np.float32)


def kernel(idx, embed, Wx, Wh, b, Wout, _t_run=T):
    from concourse.bass_utils import run_bass_kernel_spmd  # noqa: PLC0415

    t_run = _t_run
    in_maps = _host_prep(idx, embed, Wx, Wh, b, Wout, t_run)
    nc = _get_nc(t_run)
    res = run_bass_kernel_spmd(nc, in_maps, core_ids=list(range(NCORES)))
    return _postprocess(res.results[0]["logits"], t_run)


# revision 15
# speedup vs baseline: 1.0541x; 1.0541x over previous
"""CharLSTM (B=128, T=256, V=256, D=1024, L=4) on 8 trn2 NeuronCores.

Tensor-parallel over the 4*D gate dimension: core j owns, for each gate
m in {i,f,g,o}, columns [m*1024 + j*128 : m*1024 + (j+1)*128].  Hence
core j also owns h/c slice j*128:(j+1)*128 of the hidden dim.

Batch-major compute layout (activation-stationary matmuls): per layer
step, z[b, 512] = sum_k hT_chunk[k].T @ W_chunk[k] with N=512 moving
free dim, so the PE streams 512 cols per instruction (8 MMs per part
instead of 32 at N=128).  The LSTM cell runs elementwise on [b, gate]
tiles; the produced h slice [b,128] is transposed to [128 d, b] via the
DMA XBAR (off the PE), staged to DRAM, and AllGathered in layer pairs
(0,1) and (2,3) so every core has the full hT for the next step.

Layer l at time t runs at tick t + SKEW*l (wavefront), which gives the
x-part consumers >=1 tick of slack so AllGather latency hides under
matmuls of other layers.

Layer-0's x-part contracts a host-built one-hot over V=256 against
G0 = embed @ Wx[0] (computed on device), i.e. 2 matmuls instead of 8.

Output projection h3 @ Wout is computed redundantly on every core per
tick; the host reads core 0's copy.
"""

import numpy as np
import ml_dtypes

B, T, V, D, L = 128, 256, 256, 1024, 4
NCORES = 8
SKEW = 2
S = 4  # hbuf time slots
BF16 = ml_dtypes.bfloat16
TMODE = "xbar"  # "xbar" (DMA transpose) or "pe" (tensor-engine transpose)


def _host_prep(idx, embed, Wx, Wh, b, Wout, t_run):
    """Build per-core input maps (numpy)."""
    nw = (t_run + 3) // 4
    idx = np.asarray(idx)
    embed = np.asarray(embed, np.float32)
    Wx = np.asarray(Wx, np.float32)
    Wh = np.asarray(Wh, np.float32)
    b = np.asarray(b, np.float32)
    Wout = np.asarray(Wout, np.float32)
    assert not np.any(b), "nonzero bias not supported by this build"

    # embt[p, k, v] = embed[v, k*128+p]
    embt = np.ascontiguousarray(
        embed.T.reshape(8, 128, V).transpose(1, 0, 2)).astype(BF16)
    # wout[p, k, v] = Wout[k*128+p, v]
    wout = np.ascontiguousarray(
        Wout.reshape(8, 128, V).transpose(1, 0, 2)).astype(BF16)
    # one-hot: oh[w, p, c, kk, bb] = (idx[bb, 4w+kk] == c*128+p)
    ids = idx[:, :t_run]  # [B, t_run]
    onehot = (ids[None, :, :] == np.arange(V)[:, None, None])  # [V, B, t]
    oh_full = onehot.reshape(2, 128, B, nw, 4)  # [c, p, b, w, kk]
    oh = np.ascontiguousarray(
        oh_full.transpose(3, 1, 0, 4, 2)).astype(BF16)  # [w, p, c, kk, bb]

    # weight rhs layout: w[l, p, k, m*128+c] = W[l, k*128+p, m*1024+j*128+c]
    wx_full = Wx.reshape(L, 8, 128, 4, 8, 128)  # [l, k, p, m, j, c]
    wh_full = Wh.reshape(L, 8, 128, 4, 8, 128)

    in_maps = []
    for j in range(NCORES):
        wx_j = np.ascontiguousarray(
            wx_full[:, :, :, :, j, :].transpose(0, 2, 1, 3, 4)
        ).reshape(L, 128, 8, 512).astype(BF16)
        wh_j = np.ascontiguousarray(
            wh_full[:, :, :, :, j, :].transpose(0, 2, 1, 3, 4)
        ).reshape(L, 128, 8, 512).astype(BF16)
        in_map = {
            "wx": wx_j,
            "wh": wh_j,
            "embt": embt,
            "wout": wout,
            "oh": oh,
        }
        if TMODE == "pe":
            in_map["ident"] = np.eye(128, dtype=BF16)
        in_maps.append(in_map)
    return in_maps


def _build(nc, tile, mybir, t_run):
    """Emit the SPMD program for one core (identical on all cores)."""
    dt = mybir.dt
    nw = (t_run + 3) // 4

    wx_ext = nc.dram_tensor("wx", [L, 128, 8, 512], dt.bfloat16, kind="ExternalInput")
    wh_ext = nc.dram_tensor("wh", [L, 128, 8, 512], dt.bfloat16, kind="ExternalInput")
    embt_ext = nc.dram_tensor("embt", [128, 8, V], dt.bfloat16, kind="ExternalInput")
    wout_ext = nc.dram_tensor("wout", [128, 8, V], dt.bfloat16, kind="ExternalInput")
    oh_ext = nc.dram_tensor("oh", [nw, 128, 2, 4, 128], dt.bfloat16, kind="ExternalInput")
    if TMODE == "pe":
        ident_ext = nc.dram_tensor("ident", [128, 128], dt.bfloat16, kind="ExternalInput")
    out_ext = nc.dram_tensor("logits", [t_run, 128, V], dt.float32, kind="ExternalOutput")

    rg = [list(range(NCORES))]
    total_ticks = t_run + SKEW * (L - 1) + 1  # last tick flushes last proj

    with tile.TileContext(nc) as tc:
        with (
            tc.tile_pool(name="const", bufs=1) as cpool,
            tc.tile_pool(name="state", bufs=1) as spool,
            tc.tile_pool(name="work", bufs=6) as wpool,
            tc.tile_pool(name="ohp", bufs=2) as ohpool,
            tc.tile_pool(name="psum", bufs=6, space="PSUM") as psum,
            tc.tile_pool(name="ccin", bufs=3, space="DRAM") as ccin_pool,
            tc.tile_pool(name="ccout", bufs=3, space="DRAM") as ccout_pool,
        ):
            # ---- resident tiles ----
            wx_s = cpool.tile([128, L, 8, 512], dt.bfloat16)
            wh_s = cpool.tile([128, L, 8, 512], dt.bfloat16)
            embt_s = cpool.tile([128, 8, V], dt.bfloat16)
            wout_s = cpool.tile([128, 8, V], dt.bfloat16)
            g0_s = cpool.tile([128, 2, 512], dt.bfloat16)
            hbuf = spool.tile([128, L, 8, S, 128], dt.bfloat16)  # (d, l, k, slot, b)
            c_s = spool.tile([128, L, 128], dt.float32)

            for l in range(L):
                nc.sync.dma_start(wx_s[:, l], wx_ext[l])
                nc.sync.dma_start(wh_s[:, l], wh_ext[l])
            nc.sync.dma_start(embt_s[:], embt_ext[:])
            nc.sync.dma_start(wout_s[:], wout_ext[:])
            if TMODE == "pe":
                ident = cpool.tile([128, 128], dt.bfloat16)
                nc.sync.dma_start(ident[:], ident_ext[:])

            # ---- G0 = embed @ Wx[0] (slice), bf16, [v_chunk 128, 512] ----
            for c in range(2):
                pg = psum.tile([128, 512], dt.float32, tag="z", name="pg")
                for k in range(8):
                    nc.tensor.matmul(
                        pg[:], embt_s[:, k, c * 128:(c + 1) * 128], wx_s[:, 0, k],
                        start=(k == 0), stop=(k == 7),
                    )
                nc.vector.tensor_copy(g0_s[:, c], pg[:])

            # ---- main loop over ticks ----
            oh_tiles = {}
            pend_a, pend_b = [], []  # deferred hbuf scatters (prev tick's AGs)

            def emit_scatters(jobs, eng):
                for ccout_p, lp, tp in jobs:
                    # ccout[r, lp%2, p, b] -> hbuf[p, lp, r, slot, b]
                    eng.dma_start(
                        hbuf[:, lp, :, tp % S, :],
                        ccout_p[:, lp % 2, :, :].transpose((1, 0, 2)),
                    )
                jobs.clear()

            for tau in range(total_ticks):
                active = [(l, tau - SKEW * l) for l in range(L)]
                active = [(l, t) for (l, t) in active if 0 <= t < t_run]
                act_map = dict(active)

                ccinA = ccin_pool.tile([2, 128, 128], dt.bfloat16, name="ccinA")
                ccinB = ccin_pool.tile([2, 128, 128], dt.bfloat16, name="ccinB")
                ccoutA = ccout_pool.tile([NCORES, 2, 128, 128], dt.bfloat16,
                                         addr_space="Shared", name="ccoutA")
                ccoutB = ccout_pool.tile([NCORES, 2, 128, 128], dt.bfloat16,
                                         addr_space="Shared", name="ccoutB")

                # pair-A scatters from last tick (AG-A long since complete)
                emit_scatters(pend_a, nc.sync)

                # one-hot window prefetch for layer 0 (2 ticks ahead of use)
                for w in range(nw):
                    if max(0, w * 4 - 2) == tau:
                        oht = ohpool.tile([128, 2, 4, 128], dt.bfloat16, name="oh_t")
                        nc.sync.dma_start(oht[:], oh_ext[w])
                        oh_tiles[w] = oht
                oh_t = oh_tiles.get(tau // 4)

                # ---- phase A: x-part matmuls of all active layers ----
                zps = {}
                for l, t in active:
                    zp = psum.tile([128, 512], dt.float32, name="zp", tag="z")
                    zps[l] = zp
                    if l == 0:
                        for c in range(2):
                            nc.tensor.matmul(
                                zp[:], oh_t[:, c, t % 4, :], g0_s[:, c],
                                start=(c == 0), stop=(t == 0 and c == 1),
                            )
                    else:
                        for k in range(8):
                            nc.tensor.matmul(
                                zp[:], hbuf[:, l - 1, k, t % S, :], wx_s[:, l, k],
                                start=(k == 0), stop=(t == 0 and k == 7),
                            )

                # ---- projection of layer-3 output (slot gathered >=1 tick ago) ----
                t3p = tau - SKEW * 3 - 1

                def emit_proj():
                    if not (0 <= t3p < t_run):
                        return
                    pp = psum.tile([128, V], dt.float32, name="pp", tag="pp", bufs=2)
                    for k in range(8):
                        nc.tensor.matmul(
                            pp[:], hbuf[:, 3, k, t3p % S, :], wout_s[:, k],
                            start=(k == 0), stop=(k == 7),
                        )
                    lg = wpool.tile([128, V], dt.float32, name="lg")
                    nc.vector.tensor_copy(lg[:], pp[:])
                    nc.scalar.dma_start(out_ext[t3p], lg[:])

                # ---- phase B: per layer h-part + cell + stage; post pair AGs ----
                lastA = max([l for l, _ in active if l < 2], default=None)
                lastB = max([l for l, _ in active if l >= 2], default=None)
                proj_done = False
                firstB = min([l for l, _ in active if l >= 2], default=None)
                for l, t in active:
                    if l == firstB:
                        # pair-B scatters from last tick; AG-B completes ~40%
                        # into this tick, well before h2/h3's matmuls need them
                        emit_scatters(pend_b, nc.scalar)
                        emit_proj()
                        proj_done = True
                    zp = zps[l]
                    if t > 0:
                        for k in range(8):
                            nc.tensor.matmul(
                                zp[:], hbuf[:, l, k, (t - 1) % S, :], wh_s[:, l, k],
                                start=False, stop=(k == 7),
                            )

                    # ---- LSTM cell elementwise ([b, gate] layout) ----
                    # z slices: i=[0:128] f=[128:256] g=[256:384] o=[384:512]
                    sig = wpool.tile([128, 384], dt.float32, name="sig")
                    tg = wpool.tile([128, 128], dt.float32, name="tg")
                    nc.scalar.activation(
                        sig[:, 0:256], zp[:, 0:256],
                        mybir.ActivationFunctionType.Sigmoid)
                    nc.scalar.activation(
                        tg[:], zp[:, 256:384], mybir.ActivationFunctionType.Tanh)
                    nc.scalar.activation(
                        sig[:, 256:384], zp[:, 384:512],
                        mybir.ActivationFunctionType.Sigmoid)
                    ig = wpool.tile([128, 128], dt.float32, name="ig")
                    nc.vector.tensor_mul(ig[:], sig[:, 0:128], tg[:])
                    cv = c_s[:, l]
                    if t > 0:
                        nc.vector.tensor_mul(cv, cv, sig[:, 128:256])
                        nc.vector.tensor_add(cv, cv, ig[:])
                    else:
                        nc.vector.tensor_copy(cv, ig[:])
                    tch = wpool.tile([128, 128], dt.float32, name="tch")
                    nc.scalar.activation(tch[:], cv, mybir.ActivationFunctionType.Tanh)
                    h_sl = wpool.tile([128, 128], dt.bfloat16, name="h_sl")
                    nc.vector.tensor_mul(h_sl[:], sig[:, 256:384], tch[:])

                    # ---- transpose h slice to [d, b] and stage for gather ----
                    if TMODE == "xbar":
                        hT_sl = wpool.tile([128, 128], dt.bfloat16, name="hT_sl")
                        nc.sync.dma_start(hT_sl[:], h_sl[:], transpose=True)
                    else:
                        pt = psum.tile([128, 128], dt.bfloat16, name="pt", tag="pp",
                                       bufs=2)
                        nc.tensor.transpose(pt[:], h_sl[:], ident[:])
                        hT_sl = wpool.tile([128, 128], dt.bfloat16, name="hT_sl")
                        nc.vector.tensor_copy(hT_sl[:], pt[:])
                    nc.sync.dma_start((ccinA if l < 2 else ccinB)[l % 2], hT_sl[:])

                    # post the pair AllGather as soon as its layers are staged
                    if l == lastA or l == lastB:
                        ccin_p = ccinA if l == lastA else ccinB
                        ccout_p = ccoutA if l == lastA else ccoutB
                        nc.gpsimd.collective_compute(
                            "AllGather", mybir.AluOpType.bypass,
                            replica_groups=rg, ins=[ccin_p[:]], outs=[ccout_p[:]],
                        )
                        pend = pend_a if l == lastA else pend_b
                        for lp in ((0, 1) if l == lastA else (2, 3)):
                            if lp not in act_map:
                                continue
                            pend.append((ccout_p, lp, act_map[lp]))
                if not proj_done:
                    emit_scatters(pend_b, nc.scalar)
                    emit_proj()

    nc.compile()
    return nc


_CACHED = {}


def _get_nc(t_run):
    if t_run in _CACHED:
        return _CACHED[t_run]
    import concourse.bass as bass  # noqa: PLC0415
    import concourse.tile as tile  # noqa: PLC0415
    from concourse import bacc, mybir  # noqa: PLC0415

    nc = bacc.Bacc("TRN2", target_bir_lowering=False, debug=False,
                   num_devices=NCORES)
    _build(nc, tile, mybir, t_run)
    _CACHED[t_run] = nc
    return nc


def _postprocess(out, t_run):
    # out: [t, b, v] fp32 -> [B, t, V]
    return np.ascontiguousarray(
        np.asarray(out).transpose(1, 0, 2)).astype(np.float32)


def kernel(idx, embed, Wx, Wh, b, Wout, _t_run=T):
    from concourse.bass_utils import run_bass_kernel_spmd  # noqa: PLC0415

    t_run = _t_run
    in_maps = _host_prep(idx, embed, Wx, Wh, b, Wout, t_run)
    nc = _get_nc(t_run)
    res = run_bass_kernel_spmd(nc, in_maps, core_ids=list(range(NCORES)))
    return _postprocess(res.results[0]["logits"], t_run)


# revision 17
# speedup vs baseline: 1.2229x; 1.1602x over previous
"""CharLSTM (B=128, T=256, V=256, D=1024, L=4) on 8 trn2 NeuronCores.

Tensor-parallel over the 4*D gate dimension: core j owns, for each gate
m in {i,f,g,o}, columns [m*1024 + j*128 : m*1024 + (j+1)*128].  Hence
core j also owns h/c slice j*128:(j+1)*128 of the hidden dim.

Batch-major compute layout (activation-stationary matmuls): per layer
step, z[b, 512] = sum_k hT_chunk[k].T @ W_chunk[k] with N=512 moving
free dim, so the PE streams 512 cols per instruction (8 MMs per part
instead of 32 at N=128).  The LSTM cell runs elementwise on [b, gate]
tiles; the produced h slice [b,128] is transposed to [128 d, b] via the
DMA XBAR (off the PE), staged to DRAM, and AllGathered in layer pairs
(0,1) and (2,3) so every core has the full hT for the next step.

Layer l at time t runs at tick t + SKEW*l (wavefront), which gives the
x-part consumers >=1 tick of slack so AllGather latency hides under
matmuls of other layers.

Layer-0's x-part contracts a host-built one-hot over V=256 against
G0 = embed @ Wx[0] (computed on device), i.e. 2 matmuls instead of 8.

Output projection h3 @ Wout is computed redundantly on every core per
tick; the host reads core 0's copy.
"""

import numpy as np
import ml_dtypes

B, T, V, D, L = 128, 256, 256, 1024, 4
NCORES = 8
SKEW = 2
S = 4  # hbuf time slots
BF16 = ml_dtypes.bfloat16
TMODE = "pe"  # "xbar" (DMA transpose) or "pe" (tensor-engine transpose)


def _host_prep(idx, embed, Wx, Wh, b, Wout, t_run):
    """Build per-core input maps (numpy)."""
    nw = (t_run + 3) // 4
    idx = np.asarray(idx)
    embed = np.asarray(embed, np.float32)
    Wx = np.asarray(Wx, np.float32)
    Wh = np.asarray(Wh, np.float32)
    b = np.asarray(b, np.float32)
    Wout = np.asarray(Wout, np.float32)
    assert not np.any(b), "nonzero bias not supported by this build"

    # embt[p, k, v] = embed[v, k*128+p]
    embt = np.ascontiguousarray(
        embed.T.reshape(8, 128, V).transpose(1, 0, 2)).astype(BF16)
    # wout[p, k, v] = Wout[k*128+p, v]
    wout = np.ascontiguousarray(
        Wout.reshape(8, 128, V).transpose(1, 0, 2)).astype(BF16)
    # one-hot: oh[w, p, c, kk, bb] = (idx[bb, 4w+kk] == c*128+p)
    ids = idx[:, :t_run]  # [B, t_run]
    onehot = (ids[None, :, :] == np.arange(V)[:, None, None])  # [V, B, t]
    oh_full = onehot.reshape(2, 128, B, nw, 4)  # [c, p, b, w, kk]
    oh = np.ascontiguousarray(
        oh_full.transpose(3, 1, 0, 4, 2)).astype(BF16)  # [w, p, c, kk, bb]

    # weight rhs layout: w[l, p, k, m*128+c] = W[l, k*128+p, m*1024+j*128+c]
    wx_full = Wx.reshape(L, 8, 128, 4, 8, 128)  # [l, k, p, m, j, c]
    wh_full = Wh.reshape(L, 8, 128, 4, 8, 128)

    in_maps = []
    for j in range(NCORES):
        wx_j = np.ascontiguousarray(
            wx_full[:, :, :, :, j, :].transpose(0, 2, 1, 3, 4)
        ).reshape(L, 128, 8, 512).astype(BF16)
        wh_j = np.ascontiguousarray(
            wh_full[:, :, :, :, j, :].transpose(0, 2, 1, 3, 4)
        ).reshape(L, 128, 8, 512).astype(BF16)
        in_map = {
            "wx": wx_j,
            "wh": wh_j,
            "embt": embt,
            "wout": wout,
            "oh": oh,
        }
        if TMODE == "pe":
            in_map["ident"] = np.eye(128, dtype=BF16)
        in_maps.append(in_map)
    return in_maps


def _build(nc, tile, mybir, t_run):
    """Emit the SPMD program for one core (identical on all cores)."""
    dt = mybir.dt
    nw = (t_run + 3) // 4

    wx_ext = nc.dram_tensor("wx", [L, 128, 8, 512], dt.bfloat16, kind="ExternalInput")
    wh_ext = nc.dram_tensor("wh", [L, 128, 8, 512], dt.bfloat16, kind="ExternalInput")
    embt_ext = nc.dram_tensor("embt", [128, 8, V], dt.bfloat16, kind="ExternalInput")
    wout_ext = nc.dram_tensor("wout", [128, 8, V], dt.bfloat16, kind="ExternalInput")
    oh_ext = nc.dram_tensor("oh", [nw, 128, 2, 4, 128], dt.bfloat16, kind="ExternalInput")
    if TMODE == "pe":
        ident_ext = nc.dram_tensor("ident", [128, 128], dt.bfloat16, kind="ExternalInput")
    out_ext = nc.dram_tensor("logits", [t_run, 128, V], dt.float32, kind="ExternalOutput")

    rg = [list(range(NCORES))]
    total_ticks = t_run + SKEW * (L - 1) + 1  # last tick flushes last proj

    with tile.TileContext(nc) as tc:
        with (
            tc.tile_pool(name="const", bufs=1) as cpool,
            tc.tile_pool(name="state", bufs=1) as spool,
            tc.tile_pool(name="work", bufs=6) as wpool,
            tc.tile_pool(name="ohp", bufs=2) as ohpool,
            tc.tile_pool(name="psum", bufs=6, space="PSUM") as psum,
            tc.tile_pool(name="ccin", bufs=3, space="DRAM") as ccin_pool,
            tc.tile_pool(name="ccout", bufs=3, space="DRAM") as ccout_pool,
        ):
            # ---- resident tiles ----
            wx_s = cpool.tile([128, L, 8, 512], dt.bfloat16)
            wh_s = cpool.tile([128, L, 8, 512], dt.bfloat16)
            embt_s = cpool.tile([128, 8, V], dt.bfloat16)
            wout_s = cpool.tile([128, 8, V], dt.bfloat16)
            g0_s = cpool.tile([128, 2, 512], dt.bfloat16)
            hbuf = spool.tile([128, L, 8, S, 128], dt.bfloat16)  # (d, l, k, slot, b)
            c_s = spool.tile([128, L, 128], dt.float32)

            for l in range(L):
                nc.sync.dma_start(wx_s[:, l], wx_ext[l])
                nc.sync.dma_start(wh_s[:, l], wh_ext[l])
            nc.sync.dma_start(embt_s[:], embt_ext[:])
            nc.sync.dma_start(wout_s[:], wout_ext[:])
            if TMODE == "pe":
                ident = cpool.tile([128, 128], dt.bfloat16)
                nc.sync.dma_start(ident[:], ident_ext[:])

            # ---- G0 = embed @ Wx[0] (slice), bf16, [v_chunk 128, 512] ----
            for c in range(2):
                pg = psum.tile([128, 512], dt.float32, tag="z", name="pg")
                for k in range(8):
                    nc.tensor.matmul(
                        pg[:], embt_s[:, k, c * 128:(c + 1) * 128], wx_s[:, 0, k],
                        start=(k == 0), stop=(k == 7),
                    )
                nc.vector.tensor_copy(g0_s[:, c], pg[:])

            # ---- main loop over ticks ----
            oh_tiles = {}
            pend_a, pend_b = [], []  # deferred hbuf scatters (prev tick's AGs)

            def emit_scatters(jobs, eng):
                for ccout_p, lp, tp in jobs:
                    # ccout[r, lp%2, p, b] -> hbuf[p, lp, r, slot, b]
                    eng.dma_start(
                        hbuf[:, lp, :, tp % S, :],
                        ccout_p[:, lp % 2, :, :].transpose((1, 0, 2)),
                    )
                jobs.clear()

            for tau in range(total_ticks):
                active = [(l, tau - SKEW * l) for l in range(L)]
                active = [(l, t) for (l, t) in active if 0 <= t < t_run]
                act_map = dict(active)

                ccinA = ccin_pool.tile([2, 128, 128], dt.bfloat16, name="ccinA")
                ccinB = ccin_pool.tile([2, 128, 128], dt.bfloat16, name="ccinB")
                ccoutA = ccout_pool.tile([NCORES, 2, 128, 128], dt.bfloat16,
                                         addr_space="Shared", name="ccoutA")
                ccoutB = ccout_pool.tile([NCORES, 2, 128, 128], dt.bfloat16,
                                         addr_space="Shared", name="ccoutB")

                # pair-A scatters from last tick (AG-A long since complete)
                emit_scatters(pend_a, nc.sync)

                # one-hot window prefetch for layer 0 (2 ticks ahead of use)
                for w in range(nw):
                    if max(0, w * 4 - 2) == tau:
                        oht = ohpool.tile([128, 2, 4, 128], dt.bfloat16, name="oh_t")
                        nc.sync.dma_start(oht[:], oh_ext[w])
                        oh_tiles[w] = oht
                oh_t = oh_tiles.get(tau // 4)

                # ---- phase A: x-part matmuls of all active layers ----
                zps = {}
                for l, t in active:
                    zp = psum.tile([128, 512], dt.float32, name="zp", tag="z")
                    zps[l] = zp
                    if l == 0:
                        for c in range(2):
                            nc.tensor.matmul(
                                zp[:], oh_t[:, c, t % 4, :], g0_s[:, c],
                                start=(c == 0), stop=(t == 0 and c == 1),
                            )
                    else:
                        for k in range(8):
                            nc.tensor.matmul(
                                zp[:], hbuf[:, l - 1, k, t % S, :], wx_s[:, l, k],
                                start=(k == 0), stop=(t == 0 and k == 7),
                            )

                # ---- projection of layer-3 output (slot gathered >=1 tick ago) ----
                t3p = tau - SKEW * 3 - 1

                def emit_proj():
                    if not (0 <= t3p < t_run):
                        return
                    pp = psum.tile([128, V], dt.float32, name="pp", tag="pp", bufs=2)
                    for k in range(8):
                        nc.tensor.matmul(
                            pp[:], hbuf[:, 3, k, t3p % S, :], wout_s[:, k],
                            start=(k == 0), stop=(k == 7),
                        )
                    lg = wpool.tile([128, V], dt.float32, name="lg")
                    nc.vector.tensor_copy(lg[:], pp[:])
                    nc.scalar.dma_start(out_ext[t3p], lg[:])

                # ---- phase B: per layer h-part + cell + stage; post pair AGs ----
                lastA = max([l for l, _ in active if l < 2], default=None)
                lastB = max([l for l, _ in active if l >= 2], default=None)
                firstB = min([l for l, _ in active if l >= 2], default=None)
                h_tiles = {}

                def stage(l):
                    """Transpose h slice l to [d, b], stage to DRAM; post pair
                    AGs once all their layers are staged.  Called one layer
                    late so the PE transpose lands after the next layer's
                    matmuls (cell l is done by then; no PE stall)."""
                    if TMODE == "xbar":
                        hT_sl = wpool.tile([128, 128], dt.bfloat16, name="hT_sl")
                        nc.sync.dma_start(hT_sl[:], h_tiles[l][:], transpose=True)
                    else:
                        pt = psum.tile([128, 128], dt.bfloat16, name="pt", tag="pp",
                                       bufs=2)
                        nc.tensor.transpose(pt[:], h_tiles[l][:], ident[:])
                        hT_sl = wpool.tile([128, 128], dt.bfloat16, name="hT_sl")
                        nc.vector.tensor_copy(hT_sl[:], pt[:])
                    nc.sync.dma_start((ccinA if l < 2 else ccinB)[l % 2], hT_sl[:])
                    if l == lastA or l == lastB:
                        ccin_p = ccinA if l == lastA else ccinB
                        ccout_p = ccoutA if l == lastA else ccoutB
                        nc.gpsimd.collective_compute(
                            "AllGather", mybir.AluOpType.bypass,
                            replica_groups=rg, ins=[ccin_p[:]], outs=[ccout_p[:]],
                        )
                        pend = pend_a if l == lastA else pend_b
                        for lp in ((0, 1) if l == lastA else (2, 3)):
                            if lp not in act_map:
                                continue
                            pend.append((ccout_p, lp, act_map[lp]))

                for idx, (l, t) in enumerate(active):
                    if l == firstB:
                        # pair-B scatters from last tick; AG-B completes ~40%
                        # into this tick, well before h2/h3's matmuls need them
                        emit_scatters(pend_b, nc.scalar)
                        emit_proj()
                    zp = zps[l]
                    if t > 0:
                        for k in range(8):
                            nc.tensor.matmul(
                                zp[:], hbuf[:, l, k, (t - 1) % S, :], wh_s[:, l, k],
                                start=False, stop=(k == 7),
                            )
                    if idx >= 1:
                        stage(active[idx - 1][0])

                    # ---- LSTM cell elementwise ([b, gate] layout) ----
                    # z slices: i=[0:128] f=[128:256] g=[256:384] o=[384:512]
                    sig = wpool.tile([128, 384], dt.float32, name="sig")
                    tg = wpool.tile([128, 128], dt.float32, name="tg")
                    nc.scalar.activation(
                        sig[:, 0:256], zp[:, 0:256],
                        mybir.ActivationFunctionType.Sigmoid)
                    nc.scalar.activation(
                        tg[:], zp[:, 256:384], mybir.ActivationFunctionType.Tanh)
                    nc.scalar.activation(
                        sig[:, 256:384], zp[:, 384:512],
                        mybir.ActivationFunctionType.Sigmoid)
                    ig = wpool.tile([128, 128], dt.float32, name="ig")
                    nc.vector.tensor_mul(ig[:], sig[:, 0:128], tg[:])
                    cv = c_s[:, l]
                    if t > 0:
                        nc.vector.tensor_mul(cv, cv, sig[:, 128:256])
                        nc.vector.tensor_add(cv, cv, ig[:])
                    else:
                        nc.vector.tensor_copy(cv, ig[:])
                    tch = wpool.tile([128, 128], dt.float32, name="tch")
                    nc.scalar.activation(tch[:], cv, mybir.ActivationFunctionType.Tanh)
                    h_sl = wpool.tile([128, 128], dt.bfloat16, name="h_sl")
                    nc.vector.tensor_mul(h_sl[:], sig[:, 256:384], tch[:])
                    h_tiles[l] = h_sl

                if active:
                    stage(active[-1][0])
                if firstB is None:
                    emit_scatters(pend_b, nc.scalar)
                    emit_proj()

    nc.compile()
    return nc


_CACHED = {}


def _get_nc(t_run):
    if t_run in _CACHED:
        return _CACHED[t_run]
    import concourse.bass as bass  # noqa: PLC0415
    import concourse.tile as tile  # noqa: PLC0415
    from concourse import bacc, mybir  # noqa: PLC0415

    nc = bacc.Bacc("TRN2", target_bir_lowering=False, debug=False,
                   num_devices=NCORES)
    _build(nc, tile, mybir, t_run)
    _CACHED[t_run] = nc
    return nc


def _postprocess(out, t_run):
    # out: [t, b, v] fp32 -> [B, t, V]
    return np.ascontiguousarray(
        np.asarray(out).transpose(1, 0, 2)).astype(np.float32)


def kernel(idx, embed, Wx, Wh, b, Wout, _t_run=T):
    from concourse.bass_utils import run_bass_kernel_spmd  # noqa: PLC0415

    t_run = _t_run
    in_maps = _host_prep(idx, embed, Wx, Wh, b, Wout, t_run)
    nc = _get_nc(t_run)
    res = run_bass_kernel_spmd(nc, in_maps, core_ids=list(range(NCORES)))
    return _postprocess(res.results[0]["logits"], t_run)


# revision 22
# speedup vs baseline: 1.3188x; 1.0784x over previous
"""CharLSTM (B=128, T=256, V=256, D=1024, L=4) on 8 trn2 NeuronCores.

Tensor-parallel over the 4*D gate dimension: core j owns, for each gate
m in {i,f,g,o}, columns [m*1024 + j*128 : m*1024 + (j+1)*128].  Hence
core j also owns h/c slice j*128:(j+1)*128 of the hidden dim.

Batch-major compute layout (activation-stationary matmuls): per layer
step, z[b, 512] = sum_k hT_chunk[k].T @ W_chunk[k] with N=512 moving
free dim, so the PE streams 512 cols per instruction (8 MMs per part
instead of 32 at N=128).  The LSTM cell runs elementwise on [b, gate]
tiles; the produced h slice [b,128] is transposed to [128 d, b] via the
DMA XBAR (off the PE), staged to DRAM, and AllGathered in layer pairs
(0,1) and (2,3) so every core has the full hT for the next step.

Layer l at time t runs at tick t + SKEW*l (wavefront), which gives the
x-part consumers >=1 tick of slack so AllGather latency hides under
matmuls of other layers.

Layer-0's x-part contracts a host-built one-hot over V=256 against
G0 = embed @ Wx[0] (computed on device), i.e. 2 matmuls instead of 8.

Output projection h3 @ Wout is computed redundantly on every core per
tick; the host reads core 0's copy.
"""

import numpy as np
import ml_dtypes

B, T, V, D, L = 128, 256, 256, 1024, 4
NCORES = 8
SKEW = 2
S = 4  # hbuf time slots
BF16 = ml_dtypes.bfloat16
TMODE = "pe"  # "xbar" (DMA transpose) or "pe" (tensor-engine transpose)


def _host_prep(idx, embed, Wx, Wh, b, Wout, t_run):
    """Build per-core input maps (numpy)."""
    nw = (t_run + 3) // 4
    idx = np.asarray(idx)
    embed = np.asarray(embed, np.float32)
    Wx = np.asarray(Wx, np.float32)
    Wh = np.asarray(Wh, np.float32)
    b = np.asarray(b, np.float32)
    Wout = np.asarray(Wout, np.float32)
    assert not np.any(b), "nonzero bias not supported by this build"

    # embt[p, k, v] = embed[v, k*128+p]
    embt = np.ascontiguousarray(
        embed.T.reshape(8, 128, V).transpose(1, 0, 2)).astype(BF16)
    # wout[p, k, v] = Wout[k*128+p, v]
    wout = np.ascontiguousarray(
        Wout.reshape(8, 128, V).transpose(1, 0, 2)).astype(BF16)
    # one-hot: oh[w, p, c, kk, bb] = (idx[bb, 4w+kk] == c*128+p)
    ids = idx[:, :t_run]  # [B, t_run]
    onehot = (ids[None, :, :] == np.arange(V)[:, None, None])  # [V, B, t]
    oh_full = onehot.reshape(2, 128, B, nw, 4)  # [c, p, b, w, kk]
    oh = np.ascontiguousarray(
        oh_full.transpose(3, 1, 0, 4, 2)).astype(BF16)  # [w, p, c, kk, bb]

    # weight rhs layout: w[l, p, k, mm*128+c] = W[l, k*128+p, m*1024+j*128+c]
    # with gate order [i, f, o, g] (one fused 384-wide sigmoid over i,f,o)
    MSEL = [0, 1, 3, 2]
    wx_full = Wx.reshape(L, 8, 128, 4, 8, 128)[:, :, :, MSEL]  # [l, k, p, mm, j, c]
    wh_full = Wh.reshape(L, 8, 128, 4, 8, 128)[:, :, :, MSEL]

    in_maps = []
    for j in range(NCORES):
        wx_j = np.ascontiguousarray(
            wx_full[:, :, :, :, j, :].transpose(0, 2, 1, 3, 4)
        ).reshape(L, 128, 8, 512).astype(BF16)
        wh_j = np.ascontiguousarray(
            wh_full[:, :, :, :, j, :].transpose(0, 2, 1, 3, 4)
        ).reshape(L, 128, 8, 512).astype(BF16)
        in_map = {
            "wx": wx_j,
            "wh": wh_j,
            "embt": embt,
            "wout": wout,
            "oh": oh,
        }
        if TMODE == "pe":
            in_map["ident"] = np.eye(128, dtype=BF16)
        in_maps.append(in_map)
    return in_maps


def _build(nc, tile, mybir, t_run):
    """Emit the SPMD program for one core (identical on all cores)."""
    dt = mybir.dt
    nw = (t_run + 3) // 4

    wx_ext = nc.dram_tensor("wx", [L, 128, 8, 512], dt.bfloat16, kind="ExternalInput")
    wh_ext = nc.dram_tensor("wh", [L, 128, 8, 512], dt.bfloat16, kind="ExternalInput")
    embt_ext = nc.dram_tensor("embt", [128, 8, V], dt.bfloat16, kind="ExternalInput")
    wout_ext = nc.dram_tensor("wout", [128, 8, V], dt.bfloat16, kind="ExternalInput")
    oh_ext = nc.dram_tensor("oh", [nw, 128, 2, 4, 128], dt.bfloat16, kind="ExternalInput")
    if TMODE == "pe":
        ident_ext = nc.dram_tensor("ident", [128, 128], dt.bfloat16, kind="ExternalInput")
    out_ext = nc.dram_tensor("logits", [t_run, 128, V], dt.float32, kind="ExternalOutput")

    rg = [list(range(NCORES))]
    total_ticks = t_run + SKEW * (L - 1) + 1  # last tick flushes last proj

    with tile.TileContext(nc) as tc:
        with (
            tc.tile_pool(name="const", bufs=1) as cpool,
            tc.tile_pool(name="state", bufs=1) as spool,
            tc.tile_pool(name="work", bufs=6) as wpool,
            tc.tile_pool(name="ohp", bufs=2) as ohpool,
            tc.tile_pool(name="psum", bufs=6, space="PSUM") as psum,
            tc.tile_pool(name="ccin", bufs=3, space="DRAM") as ccin_pool,
            tc.tile_pool(name="ccout", bufs=3, space="DRAM") as ccout_pool,
        ):
            # ---- resident tiles ----
            wx_s = cpool.tile([128, L, 8, 512], dt.bfloat16)
            wh_s = cpool.tile([128, L, 8, 512], dt.bfloat16)
            embt_s = cpool.tile([128, 8, V], dt.bfloat16)
            wout_s = cpool.tile([128, 8, V], dt.bfloat16)
            g0_s = cpool.tile([128, 2, 512], dt.bfloat16)
            hbuf = spool.tile([128, L, 8, S, 128], dt.bfloat16)  # (d, l, k, slot, b)
            c_s = spool.tile([128, L, 128], dt.float32)

            for l in range(L):
                nc.sync.dma_start(wx_s[:, l], wx_ext[l])
                nc.sync.dma_start(wh_s[:, l], wh_ext[l])
            nc.sync.dma_start(embt_s[:], embt_ext[:])
            nc.sync.dma_start(wout_s[:], wout_ext[:])
            if TMODE == "pe":
                ident = cpool.tile([128, 128], dt.bfloat16)
                nc.sync.dma_start(ident[:], ident_ext[:])

            # ---- G0 = embed @ Wx[0] (slice), bf16, [v_chunk 128, 512] ----
            for c in range(2):
                pg = psum.tile([128, 512], dt.float32, tag="z", name="pg")
                for k in range(8):
                    nc.tensor.matmul(
                        pg[:], embt_s[:, k, c * 128:(c + 1) * 128], wx_s[:, 0, k],
                        start=(k == 0), stop=(k == 7),
                    )
                nc.vector.tensor_copy(g0_s[:, c], pg[:])

            # ---- main loop over ticks ----
            oh_tiles = {}
            pend_a, pend_b = [], []  # deferred hbuf scatters (prev tick's AGs)

            def emit_scatters(jobs, eng):
                for ccout_p, lp, tp in jobs:
                    # ccout[r, lp%2, p, b] -> hbuf[p, lp, r, slot, b]
                    eng.dma_start(
                        hbuf[:, lp, :, tp % S, :],
                        ccout_p[:, lp % 2, :, :].transpose((1, 0, 2)),
                    )
                jobs.clear()

            for tau in range(total_ticks):
                active = [(l, tau - SKEW * l) for l in range(L)]
                active = [(l, t) for (l, t) in active if 0 <= t < t_run]
                act_map = dict(active)

                ccinA = ccin_pool.tile([2, 128, 128], dt.bfloat16, name="ccinA")
                ccinB = ccin_pool.tile([2, 128, 128], dt.bfloat16, name="ccinB")
                ccoutA = ccout_pool.tile([NCORES, 2, 128, 128], dt.bfloat16,
                                         addr_space="Shared", name="ccoutA")
                ccoutB = ccout_pool.tile([NCORES, 2, 128, 128], dt.bfloat16,
                                         addr_space="Shared", name="ccoutB")

                # pair-A scatters from last tick (AG-A long since complete)
                emit_scatters(pend_a, nc.sync)

                # one-hot window prefetch for layer 0 (2 ticks ahead of use)
                for w in range(nw):
                    if max(0, w * 4 - 2) == tau:
                        oht = ohpool.tile([128, 2, 4, 128], dt.bfloat16, name="oh_t")
                        nc.sync.dma_start(oht[:], oh_ext[w])
                        oh_tiles[w] = oht
                oh_t = oh_tiles.get(tau // 4)

                # ---- phase A: x-part matmuls of all active layers ----
                zps = {}
                for l, t in active:
                    zp = psum.tile([128, 512], dt.float32, name="zp", tag="z")
                    zps[l] = zp
                    if l == 0:
                        for c in range(2):
                            nc.tensor.matmul(
                                zp[:], oh_t[:, c, t % 4, :], g0_s[:, c],
                                start=(c == 0), stop=(t == 0 and c == 1),
                            )
                    else:
                        for k in range(8):
                            nc.tensor.matmul(
                                zp[:], hbuf[:, l - 1, k, t % S, :], wx_s[:, l, k],
                                start=(k == 0), stop=(t == 0 and k == 7),
                            )

                # ---- projection of layer-3 output (slot gathered >=1 tick ago) ----
                t3p = tau - SKEW * 3 - 1

                def emit_proj():
                    if not (0 <= t3p < t_run):
                        return
                    pp = psum.tile([128, V], dt.float32, name="pp", tag="pp", bufs=2)
                    for k in range(8):
                        nc.tensor.matmul(
                            pp[:], hbuf[:, 3, k, t3p % S, :], wout_s[:, k],
                            start=(k == 0), stop=(k == 7),
                        )
                    lg = wpool.tile([128, V], dt.float32, name="lg")
                    nc.vector.tensor_copy(lg[:], pp[:])
                    nc.scalar.dma_start(out_ext[t3p], lg[:])

                # ---- phase B: per layer h-part + cell + stage; post pair AGs ----
                lastA = max([l for l, _ in active if l < 2], default=None)
                lastB = max([l for l, _ in active if l >= 2], default=None)
                firstB = min([l for l, _ in active if l >= 2], default=None)
                h_tiles = {}

                def stage(l):
                    """Transpose h slice l to [d, b], stage to DRAM; post pair
                    AGs once all their layers are staged.  High priority: the
                    scheduler must run these as soon as deps allow — they feed
                    the AllGather whose latency bounds the next tick."""
                    if TMODE == "xbar":
                        hT_sl = wpool.tile([128, 128], dt.bfloat16, name="hT_sl")
                        nc.sync.dma_start(hT_sl[:], h_tiles[l][:], transpose=True)
                    else:
                        pt = psum.tile([128, 128], dt.bfloat16, name="pt", tag="pp",
                                       bufs=2)
                        nc.tensor.transpose(pt[:], h_tiles[l][:], ident[:])
                        hT_sl = wpool.tile([128, 128], dt.bfloat16, name="hT_sl")
                        nc.vector.tensor_copy(hT_sl[:], pt[:])
                    nc.sync.dma_start((ccinA if l < 2 else ccinB)[l % 2], hT_sl[:])
                    if l == lastA or l == lastB:
                        ccin_p = ccinA if l == lastA else ccinB
                        ccout_p = ccoutA if l == lastA else ccoutB
                        nc.gpsimd.collective_compute(
                            "AllGather", mybir.AluOpType.bypass,
                            replica_groups=rg, ins=[ccin_p[:]], outs=[ccout_p[:]],
                        )
                        pend = pend_a if l == lastA else pend_b
                        for lp in ((0, 1) if l == lastA else (2, 3)):
                            if lp not in act_map:
                                continue
                            pend.append((ccout_p, lp, act_map[lp]))

                for idx, (l, t) in enumerate(active):
                    if l == firstB:
                        # pair-B scatters from last tick; AG-B completes ~40%
                        # into this tick, well before h2/h3's matmuls need them
                        emit_scatters(pend_b, nc.scalar)
                        emit_proj()
                    zp = zps[l]
                    if t > 0:
                        for k in range(8):
                            nc.tensor.matmul(
                                zp[:], hbuf[:, l, k, (t - 1) % S, :], wh_s[:, l, k],
                                start=False, stop=(k == 7),
                            )
                    if idx >= 1:
                        with tc.high_priority():
                            stage(active[idx - 1][0])

                    # ---- LSTM cell elementwise ([b, gate] layout) ----
                    # z slices: i=[0:128] f=[128:256] o=[256:384] g=[384:512]
                    sig = wpool.tile([128, 384], dt.float32, name="sig")
                    tg = wpool.tile([128, 128], dt.float32, name="tg")
                    nc.scalar.activation(
                        sig[:, 0:384], zp[:, 0:384],
                        mybir.ActivationFunctionType.Sigmoid)
                    nc.scalar.activation(
                        tg[:], zp[:, 384:512], mybir.ActivationFunctionType.Tanh)
                    ig = wpool.tile([128, 128], dt.float32, name="ig")
                    nc.vector.tensor_mul(ig[:], sig[:, 0:128], tg[:])
                    cv = c_s[:, l]
                    if t > 0:
                        nc.vector.tensor_mul(cv, cv, sig[:, 128:256])
                        nc.vector.tensor_add(cv, cv, ig[:])
                    else:
                        nc.vector.tensor_copy(cv, ig[:])
                    tch = wpool.tile([128, 128], dt.float32, name="tch")
                    nc.scalar.activation(tch[:], cv, mybir.ActivationFunctionType.Tanh)
                    h_sl = wpool.tile([128, 128], dt.bfloat16, name="h_sl")
                    nc.vector.tensor_mul(h_sl[:], sig[:, 256:384], tch[:])
                    h_tiles[l] = h_sl

                if active:
                    with tc.high_priority():
                        stage(active[-1][0])
                if firstB is None:
                    emit_scatters(pend_b, nc.scalar)
                    emit_proj()

    nc.compile()
    return nc


_CACHED = {}


def _get_nc(t_run):
    if t_run in _CACHED:
        return _CACHED[t_run]
    import concourse.bass as bass  # noqa: PLC0415
    import concourse.tile as tile  # noqa: PLC0415
    from concourse import bacc, mybir  # noqa: PLC0415

    nc = bacc.Bacc("TRN2", target_bir_lowering=False, debug=False,
                   num_devices=NCORES)
    _build(nc, tile, mybir, t_run)
    _CACHED[t_run] = nc
    return nc


def _postprocess(out, t_run):
    # out: [t, b, v] fp32 -> [B, t, V]
    return np.ascontiguousarray(
        np.asarray(out).transpose(1, 0, 2)).astype(np.float32)


def kernel(idx, embed, Wx, Wh, b, Wout, _t_run=T):
    from concourse.bass_utils import run_bass_kernel_spmd  # noqa: PLC0415

    t_run = _t_run
    in_maps = _host_prep(idx, embed, Wx, Wh, b, Wout, t_run)
    nc = _get_nc(t_run)
    res = run_bass_kernel_spmd(nc, in_maps, core_ids=list(range(NCORES)))
    return _postprocess(res.results[0]["logits"], t_run)
